# revision 15
# baseline (speedup 1.0000x reference)
"""ConvLSTM (reduces to plain LSTM: conv over length-1 axis -> only middle tap).

Strategy: data-parallel over batch across 8 NeuronCores (B_local = 8/core).
  Phase 1 (bulk, parallel over time): gates_x = Wx @ x + b for all steps,
          stored bf16 in DRAM, gate-major layout (values pre-scaled by SC).
  Phase 2 (sequential scan over S=2048): per step one identity-weight matmul
          injects gates_x into a single [128,128] PSUM tile, then 64 fp8
          Wh-matmuls (weights pre-scaled by SC) accumulate the recurrent
          term. ACT reads PSUM directly with scale=1/SC (sigmoid/tanh),
          DVE does the 3 cell ops, h is written straight into the output
          staging tile (no copy).

Layouts (per core):
  Gate rows reordered to [g, i, f, o] blocks of 512 (ref order i,f,o,g).
  M-chunk m in 0..15: reordered gate rows m*128..m*128+127 (gamma = m//4, j = m%4).
  hidden unit u = 128*q + p lives at partition p, free-slot q.
  h tiles: [128, 32] with col = q*8 + b_local.
  PSUM per step: [128, 128] = [g(0:32) | i(32:64) | f(64:96) | o(96:128)],
  within a gate: col = j*8 + b_local.
"""

import sys
import numpy as np

for _p in ("/opt/trn_rl_repo",):
    if _p not in sys.path:
        sys.path.append(_p)

import concourse.bass as bass
import concourse.mybir as mybir
from concourse.tile import TileContext
from concourse import bass_utils
from ml_dtypes import bfloat16, float8_e3m4

AF = mybir.ActivationFunctionType
FP32 = mybir.dt.float32
BF16 = mybir.dt.bfloat16
FP8E3 = mybir.dt.float8e3
SC = 512.0            # fp8 weight pre-scale (2^9); descaled in ACT scale
ISC = 1.0 / SC

B, CIN, S, HC = 64, 256, 2048, 512
NCORES = 8
BL = B // NCORES          # 8 batch per core
G4 = 4 * HC               # 2048 gate rows
T = 256                   # steps per For_i block
NBLK = S // T
NTOK = BL * S             # 16384 tokens per core
TOKB = 512                # tokens per precompute matmul
NTB = NTOK // TOKB        # 32 token blocks
# ref gate row order [i, f, o, g]; ours [g, i, f, o]
GPERM = np.concatenate([np.arange(1536, 2048), np.arange(0, 512),
                        np.arange(512, 1024), np.arange(1024, 1536)])


def _split_multiwaits(nc):
    """This walrus build allows only ONE sync-wait command per instruction.
    Hoist extra waits onto single-wait NoOps on the same engine stream."""
    nnop = 0
    for f in nc.m.functions:
        for blk in f.blocks:
            newl = []
            dirty = False
            for inst in blk.instructions:
                si = inst.sync_info
                if si and si.on_wait and len(si.on_wait) > 1:
                    waits = list(si.on_wait)
                    for w in waits[:-1]:
                        nop = mybir.InstNoOp(name=f"wsplit-{nnop}")
                        nnop += 1
                        nop.engine = inst.engine
                        nop.sync_info = mybir.SyncInfo(on_wait=[w], on_update=[])
                        newl.append(nop)
                    inst.sync_info = mybir.SyncInfo(
                        on_wait=[waits[-1]], on_update=list(si.on_update))
                    dirty = True
                newl.append(inst)
            if dirty:
                blk.instructions = newl
    return nnop


def build_nc():
    nc = bass.Bass()
    x_d = nc.dram_tensor("x", [128, 2, S, BL], BF16, kind="ExternalInput")
    whT_d = nc.dram_tensor("whT", [128, 4, G4], FP8E3, kind="ExternalInput")
    wxT_d = nc.dram_tensor("wxT", [128, 2, G4], BF16, kind="ExternalInput")
    b_d = nc.dram_tensor("bias", [128, 16], FP32, kind="ExternalInput")
    id_d = nc.dram_tensor("ident", [128, 128], BF16, kind="ExternalInput")
    gx_d = nc.dram_tensor("gx", [4, 128, S, 4, BL], BF16, kind="Internal")
    out_d = nc.dram_tensor("out", [128, S, 4, BL], BF16, kind="ExternalOutput")

    with TileContext(nc) as tc:
        with (
            tc.tile_pool(name="const", bufs=1) as cpool,
            tc.tile_pool(name="state", bufs=1) as spool,
        ):
            whT = cpool.tile([128, 4 * G4], FP8E3)
            wxT = cpool.tile([128, 2 * G4], BF16)
            bias = cpool.tile([128, 16], FP32)
            ident = cpool.tile([128, 128], BF16)
            nc.sync.dma_start(out=whT[:, :], in_=whT_d[:, :, :])
            nc.sync.dma_start(out=wxT[:, :], in_=wxT_d[:, :, :])
            nc.sync.dma_start(out=bias[:, :], in_=b_d[:, :])
            nc.sync.dma_start(out=ident[:, :], in_=id_d[:, :])

            h_st = spool.tile([128, 32], BF16)
            gc = spool.tile([128, 64], FP32)  # [tanh_g | c] side by side
            nc.vector.memset(h_st[:, :], 0.0)
            nc.vector.memset(gc[:, :], 0.0)

            # ---------------- Phase 1: gates_x precompute ----------------
            with (
                tc.tile_pool(name="xin", bufs=3) as xpool,
                tc.tile_pool(name="pcps", bufs=4, space="PSUM") as pcps,
                tc.tile_pool(name="gxe", bufs=4) as gxep,
            ):
                TS = TOKB // BL  # 64 steps per token block
                for tb in range(NTB):
                    t0 = tb * TS
                    xt = [xpool.tile([128, TOKB], BF16, tag=f"x{k}", name=f"xt{k}") for k in range(2)]
                    for k in range(2):
                        nc.sync.dma_start(
                            out=xt[k][:, :],
                            in_=x_d[:, k, t0:t0 + TS, :])
                    for g in range(4):
                        # stage all 4 j-chunks of gate g in (t, j, b) order so
                        # the DRAM write is one fully-contiguous burst per row
                        ge = gxep.tile([128, 4 * TOKB], BF16, tag="ge")
                        gev = ge.rearrange("p (t j b) -> p t j b", t=TS, j=4, b=BL)
                        for j in range(4):
                            m = g * 4 + j
                            ps = pcps.tile([128, TOKB], FP32, tag="pc")
                            for k in range(2):
                                nc.tensor.matmul(
                                    ps[:, :],
                                    wxT[:, k * G4 + m * 128: k * G4 + (m + 1) * 128],
                                    xt[k][:, :],
                                    start=(k == 0), stop=(k == 1))
                            if j % 2 == 0:
                                nc.scalar.activation(
                                    out=gev[:, :, j, :], in_=ps[:, :],
                                    func=AF.Identity, bias=bias[:, m:m + 1])
                            else:
                                nc.vector.tensor_scalar_add(
                                    out=gev[:, :, j, :], in0=ps[:, :],
                                    scalar1=bias[:, m:m + 1])
                        nc.gpsimd.dma_start(
                            out=gx_d[g, :, t0:t0 + TS, :, :],
                            in_=ge[:, :])

            # DRAM (gx_d) RAW across phases is not tracked by Tile -> hard barrier
            tc.strict_bb_all_engine_barrier()

            # ---------------- Phase 2: recurrence ----------------
            with (
                tc.tile_pool(name="gxin", bufs=2) as gxp,
                tc.tile_pool(name="obuf", bufs=2) as obp,
                tc.tile_pool(name="rps", bufs=6, space="PSUM") as rps,
                tc.tile_pool(name="work", bufs=6) as wk,
            ):
                TH = T // 2

                def load_gx(b):
                    """issue the 8 gx DMAs for block b; returns tile pair"""
                    t0 = b * T
                    tiles = [gxp.tile([128, TH * 128], BF16, tag=f"gx{h}",
                                      name=f"gxt{h}_{b}") for h in range(2)]
                    for h in range(2):
                        gv = tiles[h].rearrange("p (t g j b) -> p t g j b",
                                                t=TH, g=4, j=4, b=BL)
                        for g in range(4):
                            nc.sync.dma_start(
                                out=gv[:, :, g, :, :],
                                in_=gx_d[g, :, t0 + h * TH:t0 + (h + 1) * TH, :, :])
                    return tiles

                gxt_next = load_gx(0)
                ob_prev = None
                for b in range(NBLK):
                    gxt = gxt_next
                    ob = obp.tile([128, T * 32], BF16, tag="ob")

                    for t in range(T):
                        if t == 24 and b + 1 < NBLK:
                            gxt_next = load_gx(b + 1)
                        # h source: previous step's output column
                        if t == 0 and b == 0:
                            def hs(k):
                                return h_st[:, k * 8:(k + 1) * 8]
                        elif t == 0:
                            def hs(k, _o=ob_prev):
                                return _o[:, (T - 1) * 32 + k * 8:
                                          (T - 1) * 32 + (k + 1) * 8]
                        else:
                            def hs(k, _t=t, _o=ob):
                                return _o[:, (_t - 1) * 32 + k * 8:
                                          (_t - 1) * 32 + (k + 1) * 8]
                        ps = rps.tile([128, 128], FP32, tag="ps", name="ps")
                        # per-group: identity matmul injects gates_x (start),
                        # then fp8 Wh matmuls accumulate; stop per group so
                        # each gate's ACT can fire as soon as its cols land.
                        groups = [(0, [0]), (32, [1, 2]), (96, [3])]
                        gxb = gxt[t // TH]
                        gx0 = (t % TH) * 128
                        for c0, gates in groups:
                            nw = len(gates) * 32
                            nc.tensor.matmul(ps[:, c0:c0 + nw], ident[:, :],
                                             gxb[:, gx0 + c0:gx0 + c0 + nw],
                                             start=True, stop=False,
                                             skip_group_check=True)
                            for gi, g in enumerate(gates):
                                for j in range(4):
                                    m = g * 4 + j
                                    cj = c0 + gi * 32 + j * 8
                                    for k in range(4):
                                        nc.tensor.matmul(
                                            ps[:, cj:cj + 8],
                                            whT[:, k * G4 + m * 128: k * G4 + (m + 1) * 128],
                                            hs(k),
                                            start=False,
                                            stop=(gi == len(gates) - 1 and j == 3 and k == 3),
                                            skip_group_check=True)
                        acif = wk.tile([128, 64], FP32, tag="acif")
                        aco = wk.tile([128, 32], FP32, tag="aco")
                        igfc = wk.tile([128, 64], FP32, tag="igfc")
                        tc_ = wk.tile([128, 32], FP32, tag="tc")
                        # tanh(g) -> gc[:,0:32]; runs as soon as g group stops
                        nc.scalar.activation(out=gc[:, 0:32], in_=ps[:, 0:32],
                                             func=AF.Tanh, scale=ISC)
                        # sigmoid(i|f) after the if group stops (mid-burst)
                        nc.scalar.activation(out=acif[:, :], in_=ps[:, 32:96],
                                             func=AF.Sigmoid, scale=ISC)
                        # c' = i*g + f*c on DVE, overlapped with o's matmuls
                        nc.vector.tensor_mul(out=igfc[:, :], in0=acif[:, :], in1=gc[:, :])
                        nc.vector.tensor_add(out=gc[:, 32:64], in0=igfc[:, 0:32],
                                             in1=igfc[:, 32:64])
                        # tanh(c) emitted BEFORE sigmoid(o) on the ACT queue:
                        # its input is ready mid-burst, sigmoid(o) only at the end
                        nc.scalar.activation(out=tc_[:, :], in_=gc[:, 32:64], func=AF.Tanh)
                        nc.scalar.activation(out=aco[:, :], in_=ps[:, 96:128],
                                             func=AF.Sigmoid, scale=ISC)
                        nc.vector.tensor_mul(out=ob[:, t * 32:(t + 1) * 32],
                                             in0=aco[:, :], in1=tc_[:, :])
                    nc.sync.dma_start(out=out_d[:, b * T:(b + 1) * T, :, :],
                                      in_=ob[:, :])
                    ob_prev = ob
    _split_multiwaits(nc)
    return nc


def _prep_core_inputs(x_core, W, b):
    """x_core [BL, 256, S] f32 -> per-core input dict."""
    Wm = W[:, :, 1][GPERM]              # [2048, 768] reordered rows
    Wx = Wm[:, :CIN]                    # [2048, 256]
    Wh = Wm[:, CIN:]                    # [2048, 512]
    whT = np.ascontiguousarray(
        (Wh.T * SC).reshape(4, 128, G4).transpose(1, 0, 2)).astype(float8_e3m4)
    wxT = np.ascontiguousarray(
        (Wx.T * SC).reshape(2, 128, G4).transpose(1, 0, 2)).astype(bfloat16)
    bias = np.ascontiguousarray(
        (b[GPERM] * SC).reshape(16, 128).T).astype(np.float32)
    ident = np.eye(128, dtype=bfloat16)
    # x_d [128 p, 2 kc, S, BL]: x_core[b, kc*128+p, s]
    xr = np.ascontiguousarray(
        x_core.reshape(BL, 2, 128, S).transpose(2, 1, 3, 0)).astype(bfloat16)
    return {"x": xr, "whT": whT, "wxT": wxT, "bias": bias, "ident": ident}


def kernel(x, W, b):
    x = np.asarray(x, dtype=np.float32)
    W = np.asarray(W, dtype=np.float32)
    b = np.asarray(b, dtype=np.float32)
    nc = build_nc()
    in_maps = [_prep_core_inputs(x[c * BL:(c + 1) * BL], W, b)
               for c in range(NCORES)]
    res = bass_utils.run_bass_kernel_spmd(nc, in_maps, core_ids=list(range(NCORES)))
    outs = []
    for c in range(NCORES):
        o = np.asarray(res.results[c]["out"], dtype=np.float32)  # [128, S, 4, BL]
        outs.append(o.transpose(3, 2, 0, 1).reshape(BL, HC, S))
    return np.concatenate(outs, axis=0)


if __name__ == "__main__":
    d = np.load("/root/problem/ref_cache.npz")
    out = kernel(d["x"], d["W"], d["b"])
    exp = d["expected"]
    err = np.abs(out - exp).max() / (np.abs(exp).max() + 1e-9)
    print("rel err:", err)


# revision 19
# speedup vs baseline: 1.2492x; 1.2492x over previous
"""ConvLSTM (reduces to plain LSTM: conv over length-1 axis -> only middle tap).

Strategy: data-parallel over batch across 8 NeuronCores (B_local = 8/core).
  Phase 1 (bulk, parallel over time): gates_x = Wx @ x + b for all steps,
          stored bf16 in DRAM, gate-major layout (values pre-scaled by SC).
  Phase 2 (sequential scan over S=2048): per step one identity-weight matmul
          injects gates_x into a single [128,128] PSUM tile, then 64 fp8
          Wh-matmuls (weights pre-scaled by SC) accumulate the recurrent
          term. ACT reads PSUM directly with scale=1/SC (sigmoid/tanh),
          DVE does the 3 cell ops, h is written straight into the output
          staging tile (no copy).

Layouts (per core):
  Gate rows reordered to [g, i, f, o] blocks of 512 (ref order i,f,o,g).
  M-chunk m in 0..15: reordered gate rows m*128..m*128+127 (gamma = m//4, j = m%4).
  hidden unit u = 128*q + p lives at partition p, free-slot q.
  h tiles: [128, 32] with col = q*8 + b_local.
  PSUM per step: [128, 128] = [g(0:32) | i(32:64) | f(64:96) | o(96:128)],
  within a gate: col = j*8 + b_local.
"""

import sys
import numpy as np

for _p in ("/opt/trn_rl_repo",):
    if _p not in sys.path:
        sys.path.append(_p)

import concourse.bass as bass
import concourse.mybir as mybir
from concourse.tile import TileContext
from concourse import bass_utils
from ml_dtypes import bfloat16, float8_e3m4

AF = mybir.ActivationFunctionType
FP32 = mybir.dt.float32
BF16 = mybir.dt.bfloat16
FP8E3 = mybir.dt.float8e3
SC = 512.0            # fp8 weight pre-scale (2^9); descaled in ACT scale
ISC = 1.0 / SC

B, CIN, S, HC = 64, 256, 2048, 512
NCORES = 8
BL = B // NCORES          # 8 batch per core
G4 = 4 * HC               # 2048 gate rows
T = 256                   # steps per For_i block
NBLK = S // T
NTOK = BL * S             # 16384 tokens per core
TOKB = 512                # tokens per precompute matmul
NTB = NTOK // TOKB        # 32 token blocks
# ref gate row order [i, f, o, g]; ours [g, i, f, o]
GPERM = np.concatenate([np.arange(1536, 2048), np.arange(0, 512),
                        np.arange(512, 1024), np.arange(1024, 1536)])


def _split_multiwaits(nc):
    """This walrus build allows only ONE sync-wait command per instruction.
    Hoist extra waits onto single-wait NoOps on the same engine stream."""
    nnop = 0
    for f in nc.m.functions:
        for blk in f.blocks:
            newl = []
            dirty = False
            for inst in blk.instructions:
                si = inst.sync_info
                if si and si.on_wait and len(si.on_wait) > 1:
                    waits = list(si.on_wait)
                    for w in waits[:-1]:
                        nop = mybir.InstNoOp(name=f"wsplit-{nnop}")
                        nnop += 1
                        nop.engine = inst.engine
                        nop.sync_info = mybir.SyncInfo(on_wait=[w], on_update=[])
                        newl.append(nop)
                    inst.sync_info = mybir.SyncInfo(
                        on_wait=[waits[-1]], on_update=list(si.on_update))
                    dirty = True
                newl.append(inst)
            if dirty:
                blk.instructions = newl
    return nnop


def build_nc():
    nc = bass.Bass()
    x_d = nc.dram_tensor("x", [128, 2, S, BL], BF16, kind="ExternalInput")
    whT_d = nc.dram_tensor("whT", [128, 4, G4], FP8E3, kind="ExternalInput")
    wxT_d = nc.dram_tensor("wxT", [128, 2, G4], BF16, kind="ExternalInput")
    b_d = nc.dram_tensor("bias", [128, 16], FP32, kind="ExternalInput")
    id_d = nc.dram_tensor("ident", [128, 128], BF16, kind="ExternalInput")
    gx_d = nc.dram_tensor("gx", [4, 128, S, 4, BL], BF16, kind="Internal")
    out_d = nc.dram_tensor("out", [128, S, 4, BL], BF16, kind="ExternalOutput")

    with TileContext(nc) as tc:
        with (
            tc.tile_pool(name="const", bufs=1) as cpool,
            tc.tile_pool(name="state", bufs=1) as spool,
        ):
            whT = cpool.tile([128, 4 * G4], FP8E3)
            wxT = cpool.tile([128, 2 * G4], BF16)
            bias = cpool.tile([128, 16], FP32)
            ident = cpool.tile([128, 128], BF16)
            nc.sync.dma_start(out=whT[:, :], in_=whT_d[:, :, :])
            nc.sync.dma_start(out=wxT[:, :], in_=wxT_d[:, :, :])
            nc.sync.dma_start(out=bias[:, :], in_=b_d[:, :])
            nc.sync.dma_start(out=ident[:, :], in_=id_d[:, :])

            h_st = spool.tile([128, 32], BF16)
            gc = spool.tile([128, 64], FP32)  # [tanh_g | c] side by side
            nc.vector.memset(h_st[:, :], 0.0)
            nc.vector.memset(gc[:, :], 0.0)

            # ---------------- Phase 1: gates_x precompute ----------------
            with (
                tc.tile_pool(name="xin", bufs=3) as xpool,
                tc.tile_pool(name="pcps", bufs=4, space="PSUM") as pcps,
                tc.tile_pool(name="gxe", bufs=4) as gxep,
            ):
                TS = TOKB // BL  # 64 steps per token block
                for tb in range(NTB):
                    t0 = tb * TS
                    xt = [xpool.tile([128, TOKB], BF16, tag=f"x{k}", name=f"xt{k}") for k in range(2)]
                    for k in range(2):
                        nc.sync.dma_start(
                            out=xt[k][:, :],
                            in_=x_d[:, k, t0:t0 + TS, :])
                    for g in range(4):
                        # stage all 4 j-chunks of gate g in (t, j, b) order so
                        # the DRAM write is one fully-contiguous burst per row
                        ge = gxep.tile([128, 4 * TOKB], BF16, tag="ge")
                        gev = ge.rearrange("p (t j b) -> p t j b", t=TS, j=4, b=BL)
                        for j in range(4):
                            m = g * 4 + j
                            ps = pcps.tile([128, TOKB], FP32, tag="pc")
                            for k in range(2):
                                nc.tensor.matmul(
                                    ps[:, :],
                                    wxT[:, k * G4 + m * 128: k * G4 + (m + 1) * 128],
                                    xt[k][:, :],
                                    start=(k == 0), stop=(k == 1))
                            if j % 2 == 0:
                                nc.scalar.activation(
                                    out=gev[:, :, j, :], in_=ps[:, :],
                                    func=AF.Identity, bias=bias[:, m:m + 1])
                            else:
                                nc.vector.tensor_scalar_add(
                                    out=gev[:, :, j, :], in0=ps[:, :],
                                    scalar1=bias[:, m:m + 1])
                        nc.gpsimd.dma_start(
                            out=gx_d[g, :, t0:t0 + TS, :, :],
                            in_=ge[:, :])

            # DRAM (gx_d) RAW across phases is not tracked by Tile -> hard barrier
            tc.strict_bb_all_engine_barrier()

            # ---------------- Phase 2: recurrence ----------------
            with (
                tc.tile_pool(name="gxin", bufs=2) as gxp,
                tc.tile_pool(name="obuf", bufs=2) as obp,
                tc.tile_pool(name="rpsg", bufs=3, space="PSUM") as rpsg,
                tc.tile_pool(name="rpsif", bufs=3, space="PSUM") as rpsif,
                tc.tile_pool(name="rpso", bufs=2, space="PSUM") as rpso,
                tc.tile_pool(name="work", bufs=6) as wk,
            ):
                TH = T // 2

                def load_gx(b):
                    """issue the 8 gx DMAs for block b; returns tile pair"""
                    t0 = b * T
                    tiles = [gxp.tile([128, TH * 128], BF16, tag=f"gx{h}",
                                      name=f"gxt{h}_{b}") for h in range(2)]
                    for h in range(2):
                        gv = tiles[h].rearrange("p (t g j b) -> p t g j b",
                                                t=TH, g=4, j=4, b=BL)
                        for g in range(4):
                            nc.sync.dma_start(
                                out=gv[:, :, g, :, :],
                                in_=gx_d[g, :, t0 + h * TH:t0 + (h + 1) * TH, :, :])
                    return tiles

                gxt_next = load_gx(0)
                ob_prev = None
                for b in range(NBLK):
                    gxt = gxt_next
                    ob = obp.tile([128, T * 32], BF16, tag="ob")

                    for t in range(T):
                        if t == 24 and b + 1 < NBLK:
                            gxt_next = load_gx(b + 1)
                        # h source: previous step's output column
                        if t == 0 and b == 0:
                            def hs(k):
                                return h_st[:, k * 8:(k + 1) * 8]
                        elif t == 0:
                            def hs(k, _o=ob_prev):
                                return _o[:, (T - 1) * 32 + k * 8:
                                          (T - 1) * 32 + (k + 1) * 8]
                        else:
                            def hs(k, _t=t, _o=ob):
                                return _o[:, (_t - 1) * 32 + k * 8:
                                          (_t - 1) * 32 + (k + 1) * 8]
                        ps_g = rpsg.tile([128, 32], FP32, tag="psg", name="psg")
                        ps_if = rpsif.tile([128, 64], FP32, tag="psif", name="psif")
                        ps_o = rpso.tile([128, 32], FP32, tag="pso", name="pso")
                        # per-group: identity matmul injects gates_x (start),
                        # then fp8 Wh matmuls accumulate; stop per group so
                        # each gate's ACT can fire as soon as its cols land.
                        # g-group runs k-outer so the first 8 matmuls only
                        # need the first half of h (split mul_h below).
                        groups = [(ps_g, 0, [0], True), (ps_if, 32, [1, 2], False),
                                  (ps_o, 96, [3], False)]
                        gxb = gxt[t // TH]
                        gx0 = (t % TH) * 128
                        for pst, c0, gates, kouter in groups:
                            nw = len(gates) * 32
                            nc.tensor.matmul(pst[:, :], ident[:, :],
                                             gxb[:, gx0 + c0:gx0 + c0 + nw],
                                             start=True, stop=False,
                                             skip_group_check=True)
                            if kouter:
                                order = [(j, k) for k in range(4) for j in range(4)]
                            else:
                                order = [(j, k) for j in range(4) for k in range(4)]
                            last = order[-1]
                            for gi, g in enumerate(gates):
                                for j, k in order:
                                    m = g * 4 + j
                                    cj = gi * 32 + j * 8
                                    nc.tensor.matmul(
                                        pst[:, cj:cj + 8],
                                        whT[:, k * G4 + m * 128: k * G4 + (m + 1) * 128],
                                        hs(k),
                                        start=False,
                                        stop=(gi == len(gates) - 1 and (j, k) == last),
                                        skip_group_check=True)
                        acif = wk.tile([128, 64], FP32, tag="acif")
                        aco = wk.tile([128, 32], FP32, tag="aco")
                        igfc = wk.tile([128, 64], FP32, tag="igfc")
                        tc_ = wk.tile([128, 32], FP32, tag="tc")
                        # tanh(g) -> gc[:,0:32]; runs as soon as g group stops
                        nc.scalar.activation(out=gc[:, 0:32], in_=ps_g[:, :],
                                             func=AF.Tanh, scale=ISC)
                        # sigmoid(i|f) after the if group stops (mid-burst)
                        nc.scalar.activation(out=acif[:, :], in_=ps_if[:, :],
                                             func=AF.Sigmoid, scale=ISC)
                        # c' = i*g + f*c on DVE, overlapped with o's matmuls
                        nc.vector.tensor_mul(out=igfc[:, :], in0=acif[:, :], in1=gc[:, :])
                        nc.vector.tensor_add(out=gc[:, 32:64], in0=igfc[:, 0:32],
                                             in1=igfc[:, 32:64])
                        # tanh(c) emitted BEFORE sigmoid(o) on the ACT queue:
                        # its input is ready mid-burst, sigmoid(o) only at the end
                        nc.scalar.activation(out=tc_[:, :], in_=gc[:, 32:64], func=AF.Tanh)
                        nc.scalar.activation(out=aco[:, :], in_=ps_o[:, :],
                                             func=AF.Sigmoid, scale=ISC)
                        # h write split in two so the next step's k0/k1
                        # matmuls (k-outer g-group) can start on the first half
                        nc.vector.tensor_mul(out=ob[:, t * 32:t * 32 + 16],
                                             in0=aco[:, 0:16], in1=tc_[:, 0:16])
                        nc.vector.tensor_mul(out=ob[:, t * 32 + 16:(t + 1) * 32],
                                             in0=aco[:, 16:32], in1=tc_[:, 16:32])
                    nc.sync.dma_start(out=out_d[:, b * T:(b + 1) * T, :, :],
                                      in_=ob[:, :])
                    ob_prev = ob
    _split_multiwaits(nc)
    return nc


def _prep_core_inputs(x_core, W, b):
    """x_core [BL, 256, S] f32 -> per-core input dict."""
    Wm = W[:, :, 1][GPERM]              # [2048, 768] reordered rows
    Wx = Wm[:, :CIN]                    # [2048, 256]
    Wh = Wm[:, CIN:]                    # [2048, 512]
    whT = np.ascontiguousarray(
        (Wh.T * SC).reshape(4, 128, G4).transpose(1, 0, 2)).astype(float8_e3m4)
    wxT = np.ascontiguousarray(
        (Wx.T * SC).reshape(2, 128, G4).transpose(1, 0, 2)).astype(bfloat16)
    bias = np.ascontiguousarray(
        (b[GPERM] * SC).reshape(16, 128).T).astype(np.float32)
    ident = np.eye(128, dtype=bfloat16)
    # x_d [128 p, 2 kc, S, BL]: x_core[b, kc*128+p, s]
    xr = np.ascontiguousarray(
        x_core.reshape(BL, 2, 128, S).transpose(2, 1, 3, 0)).astype(bfloat16)
    return {"x": xr, "whT": whT, "wxT": wxT, "bias": bias, "ident": ident}


def kernel(x, W, b):
    x = np.asarray(x, dtype=np.float32)
    W = np.asarray(W, dtype=np.float32)
    b = np.asarray(b, dtype=np.float32)
    nc = build_nc()
    in_maps = [_prep_core_inputs(x[c * BL:(c + 1) * BL], W, b)
               for c in range(NCORES)]
    res = bass_utils.run_bass_kernel_spmd(nc, in_maps, core_ids=list(range(NCORES)))
    outs = []
    for c in range(NCORES):
        o = np.asarray(res.results[c]["out"], dtype=np.float32)  # [128, S, 4, BL]
        outs.append(o.transpose(3, 2, 0, 1).reshape(BL, HC, S))
    return np.concatenate(outs, axis=0)


if __name__ == "__main__":
    d = np.load("/root/problem/ref_cache.npz")
    out = kernel(d["x"], d["W"], d["b"])
    exp = d["expected"]
    err = np.abs(out - exp).max() / (np.abs(exp).max() + 1e-9)
    print("rel err:", err)
